# revision 1
# baseline (speedup 1.0000x reference)
"""Trainium2 Bass kernel for nn_ExpWindowAttention (windowed sparse attention).

Strategy: pure data-parallel over batch (32 -> 8 cores x 4 batches).
Host pre-transposes x to feature-major bf16 (plus a pre-gathered
window-center slice for Q) and weights to [f_in, f_out] bf16.
Per (batch, window-chunk): project K/V from a position segment (bf16 PE
matmuls, fp32 PSUM), compute dense per-head scores, extract the 11-wide
diagonal band with a flat-AP DMA gather, softmax in the compact band
domain, scatter normalized attention through a transposed DRAM staging
buffer, PE-transpose it into sheared [positions x windows] lhsT tiles,
apply attention-times-V on PE, then out-projection + 2-class head +
log-softmax. The emission order software-pipelines each iteration's
K/V projections ahead of the previous iteration's attention tail.
"""

import numpy as np
import ml_dtypes

import concourse.bass as bass
from concourse import bacc
import concourse.mybir as mybir
import concourse.tile as tile
from concourse.bass_utils import run_bass_kernel_spmd

F32 = mybir.dt.float32
BF16 = mybir.dt.bfloat16

NCORES = 8
B = 32
BL = B // NCORES          # batches per core
L = 2048
D = 1024
H = 8
HD = 128
W = 5
T = 2 * W + 1             # 11
NW = 409                  # windows per batch
NWQ = 416                 # padded center count (32-mult)
NC = 2
LP = 2176                 # padded position count (17*128)
SCALE = float(1.0 / np.sqrt(HD))
NEGLOG2 = float(-np.log(2.0))

# (window_start, window_count, seg_start, seg_len, n_tok_chunks, j_list)
CHUNKS = []
for c in range(5):
    ws = 96 * c
    if c < 4:
        wcnt, ss, sl = 96, 480 * c, 512
        jl = [(0, 24), (1, 24), (2, 24), (3, 24)]
    else:
        wcnt, ss, sl = 25, 1920, 256
        jl = [(0, 24), (1, 1)]
    CHUNKS.append((ws, wcnt, ss, sl, sl // 128, jl))
SLK = [512, 512, 512, 512, 136]  # K/scores width per chunk (tail trimmed)


def _flat_ap(t, extra_offset, dims):
    """AP over a tile's backing tensor flat element space (partition-major)."""
    return bass.AP(tensor=t.tensor, offset=t.offset + extra_offset,
                   ap=[list(d) for d in dims])


def _rowsz(t):
    """True per-partition stride (elements) of a tile, from its own AP."""
    return int(t[:].ap[0][0])


def _build(kv_bias_zero):
    nc = bacc.Bacc(None, target_bir_lowering=False)

    xt_d = nc.declare_dram_parameter("xt", [BL, D, LP], BF16, isOutput=False)
    xq_d = nc.declare_dram_parameter("xq", [BL, D, NWQ], BF16, isOutput=False)
    wqkv_d = nc.declare_dram_parameter("wqkvt", [D, 3 * D], BF16, isOutput=False)
    wo_d = nc.declare_dram_parameter("wot", [D, D], BF16, isOutput=False)
    whead_d = nc.declare_dram_parameter("wheadt", [D, NC], BF16, isOutput=False)
    bqkv_d = nc.declare_dram_parameter("bqkv", [3 * D], F32, isOutput=False)
    bo_d = nc.declare_dram_parameter("bo", [D], F32, isOutput=False)
    bhead_d = nc.declare_dram_parameter("bhead", [NC], F32, isOutput=False)
    out_d = nc.declare_dram_parameter("out", [BL, L, NC], F32, isOutput=True)

    with tile.TileContext(nc) as tc:
        import contextlib
        with contextlib.ExitStack() as ctx:
            const = ctx.enter_context(tc.tile_pool(name="const", bufs=1))
            perb = ctx.enter_context(tc.tile_pool(name="perb", bufs=2))
            perb1 = ctx.enter_context(tc.tile_pool(name="perb1", bufs=1))
            perb2 = ctx.enter_context(tc.tile_pool(name="perb2", bufs=2))
            xtp = ctx.enter_context(tc.tile_pool(name="xtp", bufs=2))
            ktp = ctx.enter_context(tc.tile_pool(name="ktp", bufs=2))
            vstp = ctx.enter_context(tc.tile_pool(name="vstp", bufs=2))
            vavp = ctx.enter_context(tc.tile_pool(name="vavp", bufs=3))
            ssbp = ctx.enter_context(tc.tile_pool(name="ssbp", bufs=3))
            smx = ctx.enter_context(tc.tile_pool(name="smx", bufs=2))
            ashp = ctx.enter_context(tc.tile_pool(name="ashp", bufs=2))
            drp = ctx.enter_context(tc.tile_pool(name="drp", bufs=3, space="DRAM"))
            proj_ps = ctx.enter_context(tc.tile_pool(name="proj_ps", bufs=2, space="PSUM"))
            tr_ps = ctx.enter_context(tc.tile_pool(name="tr_ps", bufs=1, space="PSUM"))
            sc_ps = ctx.enter_context(tc.tile_pool(name="sc_ps", bufs=3, space="PSUM"))
            av_ps = ctx.enter_context(tc.tile_pool(name="av_ps", bufs=1, space="PSUM"))

            # ---- resident weights / biases ----
            wqkv = const.tile([128, 8, 3 * D], BF16)
            nc.sync.dma_start(
                out=wqkv, in_=wqkv_d.rearrange("(kc p) c -> p kc c", p=128)
            )
            wo = const.tile([128, 8, D], BF16)
            nc.sync.dma_start(out=wo, in_=wo_d.rearrange("(kc p) c -> p kc c", p=128))
            whead = const.tile([128, 8, NC], BF16)
            nc.sync.dma_start(
                out=whead, in_=whead_d.rearrange("(kc p) c -> p kc c", p=128)
            )
            bqkv_col = const.tile([128, 24], F32)  # [p, proj*8+fc]
            nc.sync.dma_start(out=bqkv_col, in_=bqkv_d.rearrange("(c p) -> p c", p=128))
            bo_col = const.tile([128, 8], F32)
            nc.sync.dma_start(out=bo_col, in_=bo_d.rearrange("(c p) -> p c", p=128))
            bhead_col = const.tile([NC, 1], F32)
            nc.sync.dma_start(
                out=bhead_col, in_=bhead_d.rearrange("(p one) -> p one", one=1)
            )
            bv_row = const.tile([128, D], F32)  # v-bias broadcast along partitions
            nc.sync.dma_start(
                out=bv_row,
                in_=bass.AP(tensor=bqkv_d, offset=2 * D, ap=[[0, 128], [1, D]]),
            )
            cfill = const.tile([128, 8], F32)
            nc.vector.memset(cfill, NEGLOG2)
            zash = const.tile([128, 8 * 4 * 24], BF16)
            nc.vector.memset(zash, 0.0)
            ident = const.tile([128, 128], BF16)
            from concourse.masks import make_identity
            make_identity(nc, ident[:])
            ident_f = const.tile([128, 128], F32)
            make_identity(nc, ident_f[:])

            state = {}

            def qproj(b):
                """Per-batch Q projection from host-pregathered centers."""
                xq = perb1.tile([128, 8, NWQ], BF16, tag="xq")
                nc.sync.dma_start(
                    out=xq, in_=xq_d[b].rearrange("(kc p) c -> p kc c", p=128)
                )
                qt = perb.tile([128, 8, NWQ], BF16, tag="qt")
                for h in range(8):
                    qps = proj_ps.tile([128, NWQ], F32, tag="pps")
                    for kc in range(8):
                        nc.tensor.matmul(
                            qps[:],
                            wqkv[:, kc, h * 128 : h * 128 + 128],
                            xq[:, kc, :],
                            start=(kc == 0),
                            stop=(kc == 7),
                        )
                    nc.vector.tensor_scalar_add(
                        qt[:, h, :], qps[:], bqkv_col[:, h : h + 1]
                    )
                state[("qt", b)] = qt

            def produce(b, ci):
                """K/V projections for one (batch, chunk)."""
                ws, wcnt, ss, sl, ntc, jl = CHUNKS[ci]
                xt = xtp.tile([128, 8, sl], BF16, tag="xt")
                nc.sync.dma_start(
                    out=xt,
                    in_=xt_d[b].rearrange("(kc p) c -> p kc c", p=128)[
                        :, :, ss : ss + sl
                    ],
                )
                slk = SLK[ci]
                kt = ktp.tile([128, 8, slk], BF16, tag="kt")
                for h in range(8):
                    kps = proj_ps.tile([128, slk], F32, tag="pps")
                    for kc in range(8):
                        nc.tensor.matmul(
                            kps[:],
                            wqkv[:, kc, D + h * 128 : D + h * 128 + 128],
                            xt[:, kc, 0:slk],
                            start=(kc == 0),
                            stop=(kc == 7),
                        )
                    if kv_bias_zero:
                        nc.vector.tensor_copy(kt[:, h, :], kps[:])
                    else:
                        nc.vector.tensor_scalar_add(
                            kt[:, h, :], kps[:], bqkv_col[:, 8 + h : 9 + h]
                        )
                vst = vstp.tile([128, 4, D], BF16, tag="vst")
                for ti in range(ntc):
                    for half in range(2):
                        vps = proj_ps.tile([128, 512], F32, tag="pps")
                        for kc in range(8):
                            nc.tensor.matmul(
                                vps[:],
                                xt[:, kc, ti * 128 : ti * 128 + 128],
                                wqkv[
                                    :, kc,
                                    2 * D + half * 512 : 2 * D + half * 512 + 512,
                                ],
                                start=(kc == 0),
                                stop=(kc == 7),
                            )
                        if kv_bias_zero:
                            nc.scalar.copy(
                                vst[:, ti, half * 512 : half * 512 + 512], vps[:]
                            )
                        else:
                            nc.vector.tensor_tensor(
                                out=vst[:, ti, half * 512 : half * 512 + 512],
                                in0=vps[:],
                                in1=bv_row[:, half * 512 : half * 512 + 512],
                                op=mybir.AluOpType.add,
                            )
                # re-chunk V to 120-strided AV tiles
                vav = vavp.tile([128, 4, D], BF16, tag="vav")
                for j, jw in jl:
                    r0 = 120 * j
                    c0 = r0 // 128
                    o0 = r0 - c0 * 128
                    n1 = 128 - o0
                    nc.sync.dma_start(out=vav[0:n1, j, :], in_=vst[o0:128, c0, :])
                    if n1 < 128:
                        nc.sync.dma_start(
                            out=vav[n1:128, j, :], in_=vst[0 : 128 - n1, c0 + 1, :]
                        )
                state[("kt", b, ci)] = kt
                state[("vav", b, ci)] = vav

            def attend_a(b, ci):
                ws, wcnt, ss, sl, ntc, jl = CHUNKS[ci]
                qt = state[("qt", b)]
                kt = state.pop(("kt", b, ci))

                # scores per head + band gather
                slk = SLK[ci]
                band = smx.tile([wcnt, 8, T], F32, tag="band")
                for h in range(8):
                    sps = sc_ps.tile([wcnt, slk], F32, tag="sps")
                    nc.tensor.matmul(
                        sps[:], qt[:, h, ws : ws + wcnt], kt[:, h, :],
                        start=True, stop=True,
                    )
                    ssb = ssbp.tile([wcnt, slk], F32, tag="ssb")
                    if h % 2 == 0:
                        nc.scalar.copy(ssb[:], sps[:])
                    else:
                        nc.vector.tensor_copy(ssb[:], sps[:])
                    rs = _rowsz(ssb)
                    nc.sync.dma_start(
                        out=band[:, h, :],
                        in_=_flat_ap(ssb, 0, [[rs + 5, wcnt], [1, T]]),
                    )

                # softmax in band domain
                negmax = smx.tile([wcnt, 8], F32, tag="negmax")
                nc.vector.tensor_reduce(
                    negmax[:], band[:], axis=mybir.AxisListType.X,
                    op=mybir.AluOpType.max, negate=True,
                )
                negmax_s = smx.tile([wcnt, 8], F32, tag="negmax_s")
                nc.vector.tensor_scalar_mul(negmax_s[:], negmax[:], SCALE)
                eb = smx.tile([wcnt, 8, T], F32, tag="eb")
                sums = smx.tile([wcnt, 8], F32, tag="sums")
                for h in range(8):
                    nc.scalar.activation(
                        eb[:, h, :], band[:, h, :],
                        mybir.ActivationFunctionType.Exp,
                        bias=negmax_s[:, h : h + 1], scale=SCALE,
                        accum_out=sums[:, h : h + 1],
                    )
                recip = smx.tile([wcnt, 8], F32, tag="recip")
                nc.vector.reciprocal(recip[:], sums[:])
                attn = smx.tile([wcnt, 8, T], BF16, tag="attn")
                recip_bc = recip[:].unsqueeze(2).to_broadcast([wcnt, 8, T])
                nc.vector.tensor_tensor(
                    out=attn[:], in0=eb[:], in1=recip_bc, op=mybir.AluOpType.mult
                )

                # scatter attn into transposed sheared DRAM staging
                # Ash_dT[row = h*96 + j*24 + w', col = 5w' + t]; t contiguous
                ash_dT = drp.tile([768, 128], BF16, tag="ashd")
                nc.sync.dma_start(out=ash_dT[:], in_=zash[:])
                ra = _rowsz(attn)
                rdt = _rowsz(ash_dT)
                for j, jw in jl:
                    if jw == 24:
                        nc.sync.dma_start(
                            out=_flat_ap(
                                ash_dT, 24 * j * rdt,
                                [[rdt + 5, 24], [96 * rdt, 8], [1, T]],
                            ),
                            in_=_flat_ap(
                                attn, 24 * j * ra, [[ra, 24], [T, 8], [1, T]]
                            ),
                        )
                    else:
                        nc.sync.dma_start(
                            out=_flat_ap(
                                ash_dT, 24 * j * rdt, [[96 * rdt, 8], [1, T]]
                            ),
                            in_=attn[24 * j : 24 * j + 1, :, :],
                        )
                state[("ashd", b, ci)] = ash_dT

            def attend_b(b, ci):
                ws, wcnt, ss, sl, ntc, jl = CHUNKS[ci]
                vav = state.pop(("vav", b, ci))
                oT = state[("oT", b)]
                ash_dT = state.pop(("ashd", b, ci))
                # contiguous reload as [128, 6, 128], PE-transpose to [128, 768]
                ashT = ashp.tile([128, 6, 128], BF16, tag="ashT")
                nc.sync.dma_start(
                    out=ashT, in_=ash_dT[:].rearrange("(blk p) c -> p blk c", p=128)
                )
                ash = ashp.tile([128, 8 * 4 * 24], BF16, tag="ash")
                for blk in range(6):
                    tps = tr_ps.tile([128, 128], BF16, tag="ashtp")
                    nc.tensor.transpose(tps[:], ashT[:, blk, :], ident[:])
                    nc.vector.tensor_copy(ash[:, blk * 128 : blk * 128 + 128], tps[:])

                # AV: oT[d, w'] per (h, j), V stationary
                avo = av_ps.tile([128, 4, 256], F32, tag="avo")
                for j, jw in jl:
                    for h in range(8):
                        nc.tensor.matmul(
                            avo[:, j, h * 24 : h * 24 + jw],
                            vav[:, j, h * 128 : h * 128 + 128],
                            ash[:, h * 96 + j * 24 : h * 96 + j * 24 + jw],
                            start=True,
                            stop=True,
                        )
                # evict into per-batch oT at col h*NWQ + ws + 24j + w'
                if ci < 4:
                    nc.vector.tensor_copy(
                        oT[:, :, ws : ws + 96].rearrange("p h (j w) -> p j h w", j=4),
                        avo[:, :, 0:192].rearrange("p j (h w) -> p j h w", h=8),
                    )
                else:
                    nc.vector.tensor_copy(
                        oT[:, :, ws : ws + 24],
                        avo[:, 0, 0:192].rearrange("p (h w) -> p h w", h=8),
                    )
                    nc.vector.tensor_copy(
                        oT[:, :, ws + 24 : ws + 25],
                        avo[:, 1, 0:192].rearrange("p (h w) -> p h w", h=8)[:, :, 0:1],
                    )

            def tail(b):
                """out_proj + head + log_softmax + output DMA for one batch."""
                oT = state.pop(("oT", b))
                state.pop(("qt", b))
                featT = perb1.tile([128, 8, NWQ], BF16, tag="featT")
                for fc in range(8):
                    fps = proj_ps.tile([128, NWQ], F32, tag="pps")
                    for kc in range(8):
                        nc.tensor.matmul(
                            fps[:],
                            wo[:, kc, fc * 128 : fc * 128 + 128],
                            oT[:, kc, :],
                            start=(kc == 0),
                            stop=(kc == 7),
                        )
                    nc.vector.tensor_scalar_add(
                        featT[:, fc, :], fps[:], bo_col[:, fc : fc + 1]
                    )

                pps = proj_ps.tile([NC, NWQ], F32, tag="pps")
                for kc in range(8):
                    nc.tensor.matmul(
                        pps[:], whead[:, kc, :], featT[:, kc, :],
                        start=(kc == 0), stop=(kc == 7),
                    )
                probsT = perb2.tile([NC, NW], F32, tag="probsT")
                nc.vector.tensor_scalar_add(probsT[:], pps[:, 0:NW], bhead_col[:])

                # transpose probs to token-major [128, 4, 2] via PE
                ptm = perb2.tile([128, 4, NC], F32, tag="ptm")
                nc.vector.memset(ptm, 0.0)
                for wc in range(4):
                    cnt = 128 if wc < 3 else 25
                    tp2 = tr_ps.tile([128, NC], F32, tag="ashtp")
                    nc.tensor.transpose(
                        tp2[0:cnt, :], probsT[:, wc * 128 : wc * 128 + cnt],
                        ident_f[0:NC, 0:NC],
                    )
                    nc.vector.tensor_copy(ptm[0:cnt, wc, :], tp2[0:cnt, :])

                # log_softmax over the 2 classes
                pmax = perb2.tile([128, 4], F32, tag="pmax")
                nc.vector.tensor_reduce(
                    pmax[:], ptm[:], axis=mybir.AxisListType.X, op=mybir.AluOpType.max
                )
                nmax = perb2.tile([128, 4], F32, tag="nmax")
                nc.vector.tensor_scalar_mul(nmax[:], pmax[:], -1.0)
                sexp = perb2.tile([128, 4], F32, tag="sexp")
                etmp = perb2.tile([128, 4, NC], F32, tag="etmp")
                for wc in range(4):
                    nc.scalar.activation(
                        etmp[:, wc, :], ptm[:, wc, :],
                        mybir.ActivationFunctionType.Exp,
                        bias=nmax[:, wc : wc + 1], scale=1.0,
                        accum_out=sexp[:, wc : wc + 1],
                    )
                lns = perb2.tile([128, 4], F32, tag="lns")
                nc.scalar.activation(lns[:], sexp[:], mybir.ActivationFunctionType.Ln)
                otm = perb2.tile([128, 4, NC], F32, tag="otm")
                for wc in range(4):
                    nc.vector.tensor_scalar(
                        out=otm[:, wc, :],
                        in0=ptm[:, wc, :],
                        scalar1=pmax[:, wc : wc + 1],
                        scalar2=lns[:, wc : wc + 1],
                        op0=mybir.AluOpType.subtract,
                        op1=mybir.AluOpType.subtract,
                    )

                # write output: const fill + strided scatter
                nc.sync.dma_start(
                    out=bass.AP(
                        tensor=out_d, offset=b * L * NC + 2,
                        ap=[[1, 1], [10, NW], [1, 8]],
                    ),
                    in_=_flat_ap(cfill, 0, [[_rowsz(cfill), 1], [0, NW], [1, 8]]),
                )
                nc.sync.dma_start(
                    out=bass.AP(
                        tensor=out_d, offset=b * L * NC + 2045 * NC, ap=[[1, 6]]
                    ),
                    in_=cfill[0:1, 0:6],
                )
                rt = _rowsz(otm)
                nc.sync.dma_start(
                    out=bass.AP(
                        tensor=out_d, offset=b * L * NC,
                        ap=[[10, 128], [1280, 3], [1, NC]],
                    ),
                    in_=_flat_ap(otm, 0, [[rt, 128], [NC, 3], [1, NC]]),
                )
                nc.sync.dma_start(
                    out=bass.AP(
                        tensor=out_d, offset=b * L * NC + 3840, ap=[[10, 25], [1, NC]]
                    ),
                    in_=_flat_ap(otm, 3 * NC, [[rt, 25], [1, NC]]),
                )

            def start_batch(b):
                qproj(b)
                oT = perb.tile([128, 8, NWQ], BF16, tag="oT")
                nc.vector.memset(oT[:, :, NW:NWQ], 0.0)
                state[("oT", b)] = oT

            # ---- software-pipelined emission (2-deep) ----
            iters = [(b, ci) for b in range(BL) for ci in range(5)]
            start_batch(0)
            produce(0, 0)
            attend_a(0, 0)
            for i in range(1, len(iters)):
                b, ci = iters[i]
                if ci == 0:
                    start_batch(b)
                produce(b, ci)
                attend_a(b, ci)
                pb, pci = iters[i - 1]
                attend_b(pb, pci)
                if pci == 4:
                    tail(pb)
            lb, lci = iters[-1]
            attend_b(lb, lci)
            tail(lb)

    nc.compile()
    return nc


_NC_CACHE = {}


def _get_nc(kv_bias_zero):
    if kv_bias_zero not in _NC_CACHE:
        _NC_CACHE[kv_bias_zero] = _build(kv_bias_zero)
    return _NC_CACHE[kv_bias_zero]


def kernel(x, in_proj_w, in_proj_b, out_proj_w, out_proj_b, out_w, out_b, x_len=None,
           _want_perf=False):
    x = np.asarray(x, dtype=np.float32)
    in_proj_w = np.asarray(in_proj_w, dtype=np.float32)
    in_proj_b = np.asarray(in_proj_b, dtype=np.float32)
    out_proj_w = np.asarray(out_proj_w, dtype=np.float32)
    out_proj_b = np.asarray(out_proj_b, dtype=np.float32)
    out_w = np.asarray(out_w, dtype=np.float32)
    out_b = np.asarray(out_b, dtype=np.float32)

    kv_bias_zero = bool(np.all(in_proj_b[D:] == 0.0))
    nc = _get_nc(kv_bias_zero)

    # host-side layout prep
    xt = np.zeros((B, D, LP), dtype=ml_dtypes.bfloat16)
    xt[:, :, :L] = x.transpose(0, 2, 1).astype(ml_dtypes.bfloat16)
    xq = np.zeros((B, D, NWQ), dtype=ml_dtypes.bfloat16)
    xq[:, :, :NW] = xt[:, :, 5 : 5 * NW + 5 : 5]
    wqkvt = np.ascontiguousarray(in_proj_w.T).astype(ml_dtypes.bfloat16)
    wot = np.ascontiguousarray(out_proj_w.T).astype(ml_dtypes.bfloat16)
    wheadt = np.ascontiguousarray(out_w.T).astype(ml_dtypes.bfloat16)

    in_maps = []
    for c in range(NCORES):
        in_maps.append({
            "xt": np.ascontiguousarray(xt[c * BL : (c + 1) * BL]),
            "xq": np.ascontiguousarray(xq[c * BL : (c + 1) * BL]),
            "wqkvt": wqkvt,
            "wot": wot,
            "wheadt": wheadt,
            "bqkv": in_proj_b,
            "bo": out_proj_b,
            "bhead": out_b,
        })

    kr = run_bass_kernel_spmd(
        nc, in_maps, core_ids=list(range(NCORES)), trace=_want_perf
    )
    out = np.concatenate([r["out"] for r in kr.results], axis=0).reshape(-1, NC)
    if _want_perf:
        return out, kr
    return out



# revision 11
# speedup vs baseline: 1.3463x; 1.3463x over previous
"""Trainium2 Bass kernel for nn_ExpWindowAttention (windowed sparse attention).

Strategy: pure data-parallel over batch (32 -> 8 cores x 4 batches).
Host pre-transposes x to feature-major bf16 (plus a pre-gathered
window-center slice for Q) and weights to [f_in, f_out] bf16.
Per (batch, window-chunk): project K/V from a position segment (bf16 PE
matmuls, fp32 PSUM), compute dense per-head scores, extract the 11-wide
diagonal band with a flat-AP DMA gather, softmax in the compact band
domain, scatter normalized attention through a transposed DRAM staging
buffer, PE-transpose it into sheared [positions x windows] lhsT tiles,
apply attention-times-V on PE, then out-projection + 2-class head +
log-softmax. The emission order software-pipelines each iteration's
K/V projections ahead of the previous iteration's attention tail.
"""

import numpy as np
import ml_dtypes

import concourse.bass as bass
from concourse import bacc
import concourse.mybir as mybir
import concourse.tile as tile
from concourse.bass_utils import run_bass_kernel_spmd

F32 = mybir.dt.float32
BF16 = mybir.dt.bfloat16
F8 = mybir.dt.float8e4
DR = mybir.MatmulPerfMode.DoubleRow
WS8 = 128.0              # fp8 weight pre-scale for q/k/v projections

NCORES = 8
B = 32
BL = B // NCORES          # batches per core
L = 2048
D = 1024
H = 8
HD = 128
W = 5
T = 2 * W + 1             # 11
NW = 409                  # windows per batch
NWQ = 416                 # padded center count (32-mult)
NC = 2
LP = 2176                 # padded position count (17*128)
SCALE = float(1.0 / np.sqrt(HD))
SSCALE = float(SCALE / (WS8 * WS8))   # scores carry WS8^2 from fp8 q/k scaling
NEGLOG2 = float(-np.log(2.0))

# (window_start, window_count, seg_start, seg_len, n_tok_chunks, j_list)
CHUNKS = []
for c in range(5):
    ws = 96 * c
    if c < 4:
        wcnt, ss, sl = 96, 480 * c, 512
        jl = [(0, 24), (1, 24), (2, 24), (3, 24)]
    else:
        wcnt, ss, sl = 25, 1920, 256
        jl = [(0, 24), (1, 1)]
    CHUNKS.append((ws, wcnt, ss, sl, sl // 128, jl))
SLK = [512, 512, 512, 512, 136]  # K/scores width per chunk (tail trimmed)


def _flat_ap(t, extra_offset, dims):
    """AP over a tile's backing tensor flat element space (partition-major)."""
    return bass.AP(tensor=t.tensor, offset=t.offset + extra_offset,
                   ap=[list(d) for d in dims])


def _rowsz(t):
    """True per-partition stride (elements) of a tile, from its own AP."""
    return int(t[:].ap[0][0])


def _build(kv_bias_zero):
    nc = bacc.Bacc(None, target_bir_lowering=False)

    xt_d = nc.declare_dram_parameter("xt", [BL, D, LP], F8, isOutput=False)
    xq_d = nc.declare_dram_parameter("xq", [BL, D, NWQ], F8, isOutput=False)
    wqkv_d = nc.declare_dram_parameter("wqkvt", [D, 3 * D], F8, isOutput=False)
    wo_d = nc.declare_dram_parameter("wot", [D, D], BF16, isOutput=False)
    whead_d = nc.declare_dram_parameter("wheadt", [D, NC], BF16, isOutput=False)
    bqkv_d = nc.declare_dram_parameter("bqkv", [3 * D], F32, isOutput=False)
    bo_d = nc.declare_dram_parameter("bo", [D], F32, isOutput=False)
    bhead_d = nc.declare_dram_parameter("bhead", [NC], F32, isOutput=False)
    out_d = nc.declare_dram_parameter("out", [BL, L, NC], F32, isOutput=True)

    with tile.TileContext(nc) as tc:
        import contextlib
        with contextlib.ExitStack() as ctx:
            const = ctx.enter_context(tc.tile_pool(name="const", bufs=1))
            perb = ctx.enter_context(tc.tile_pool(name="perb", bufs=2))
            perb1 = ctx.enter_context(tc.tile_pool(name="perb1", bufs=1))
            perb2 = ctx.enter_context(tc.tile_pool(name="perb2", bufs=2))
            xtp = ctx.enter_context(tc.tile_pool(name="xtp", bufs=2))
            ktp = ctx.enter_context(tc.tile_pool(name="ktp", bufs=2))
            vstp = ctx.enter_context(tc.tile_pool(name="vstp", bufs=2))
            vavp = ctx.enter_context(tc.tile_pool(name="vavp", bufs=3))
            ssbp = ctx.enter_context(tc.tile_pool(name="ssbp", bufs=3))
            smx = ctx.enter_context(tc.tile_pool(name="smx", bufs=2))
            ashp = ctx.enter_context(tc.tile_pool(name="ashp", bufs=2))
            drp = ctx.enter_context(tc.tile_pool(name="drp", bufs=3, space="DRAM"))
            proj_ps = ctx.enter_context(tc.tile_pool(name="proj_ps", bufs=2, space="PSUM"))
            tr_ps = ctx.enter_context(tc.tile_pool(name="tr_ps", bufs=1, space="PSUM"))
            sc_ps = ctx.enter_context(tc.tile_pool(name="sc_ps", bufs=3, space="PSUM"))
            av_ps = ctx.enter_context(tc.tile_pool(name="av_ps", bufs=1, space="PSUM"))

            # ---- resident weights / biases ----
            wqkv = const.tile([128, 8, 3 * D], F8)
            nc.sync.dma_start(
                out=wqkv, in_=wqkv_d.rearrange("(kc p) c -> p kc c", p=128)
            )
            wo = const.tile([128, 8, D], BF16)
            nc.sync.dma_start(out=wo, in_=wo_d.rearrange("(kc p) c -> p kc c", p=128))
            whead = const.tile([128, 8, NC], BF16)
            nc.sync.dma_start(
                out=whead, in_=whead_d.rearrange("(kc p) c -> p kc c", p=128)
            )
            bqkv_col = const.tile([128, 24], F32)  # [p, proj*8+fc]
            nc.sync.dma_start(out=bqkv_col, in_=bqkv_d.rearrange("(c p) -> p c", p=128))
            bo_col = const.tile([128, 8], F32)
            nc.sync.dma_start(out=bo_col, in_=bo_d.rearrange("(c p) -> p c", p=128))
            bhead_col = const.tile([NC, 1], F32)
            nc.sync.dma_start(
                out=bhead_col, in_=bhead_d.rearrange("(p one) -> p one", one=1)
            )
            bv_row = const.tile([128, D], F32)  # v-bias broadcast along partitions
            nc.sync.dma_start(
                out=bv_row,
                in_=bass.AP(tensor=bqkv_d, offset=2 * D, ap=[[0, 128], [1, D]]),
            )
            cfill = const.tile([128, 8], F32)
            nc.vector.memset(cfill, NEGLOG2)
            zash = const.tile([128, 8 * 4 * 24], BF16)
            nc.vector.memset(zash, 0.0)
            ident = const.tile([128, 128], BF16)
            from concourse.masks import make_identity
            make_identity(nc, ident[:])
            ident_f = const.tile([128, 128], F32)
            make_identity(nc, ident_f[:])

            state = {}

            def qproj(b):
                """Per-batch Q projection from host-pregathered centers."""
                xq = perb1.tile([128, 8, NWQ], F8, tag="xq")
                nc.sync.dma_start(
                    out=xq, in_=xq_d[b].rearrange("(kc p) c -> p kc c", p=128)
                )
                qt = perb.tile([128, 8, NWQ], BF16, tag="qt")
                for h in range(8):
                    qps = proj_ps.tile([128, NWQ], F32, tag="pps")
                    for j in range(4):
                        nc.tensor.matmul(
                            qps[:],
                            wqkv[:, 2 * j : 2 * j + 2, h * 128 : h * 128 + 128],
                            xq[:, 2 * j : 2 * j + 2, :],
                            start=(j == 0),
                            stop=(j == 3),
                            perf_mode=DR,
                        )
                    nc.vector.tensor_scalar_add(
                        qt[:, h, :], qps[:], bqkv_col[:, h : h + 1]
                    )
                state[("qt", b)] = qt

            def produce(b, ci):
                """K/V projections for one (batch, chunk)."""
                ws, wcnt, ss, sl, ntc, jl = CHUNKS[ci]
                xt = xtp.tile([128, 8, sl], F8, tag="xt")
                nc.sync.dma_start(
                    out=xt,
                    in_=xt_d[b].rearrange("(kc p) c -> p kc c", p=128)[
                        :, :, ss : ss + sl
                    ],
                )
                slk = SLK[ci]
                kt = ktp.tile([128, 8, slk], BF16, tag="kt")
                for h in range(8):
                    kps = proj_ps.tile([128, slk], F32, tag="pps")
                    for j in range(4):
                        nc.tensor.matmul(
                            kps[:],
                            wqkv[:, 2 * j : 2 * j + 2, D + h * 128 : D + h * 128 + 128],
                            xt[:, 2 * j : 2 * j + 2, 0:slk],
                            start=(j == 0),
                            stop=(j == 3),
                            perf_mode=DR,
                        )
                    if kv_bias_zero:
                        nc.vector.tensor_copy(kt[:, h, :], kps[:])
                    else:
                        nc.vector.tensor_scalar_add(
                            kt[:, h, :], kps[:], bqkv_col[:, 8 + h : 9 + h]
                        )
                vst = vstp.tile([128, 4, D], BF16, tag="vst")
                for ti in range(ntc):
                    for half in range(2):
                        vps = proj_ps.tile([128, 512], F32, tag="pps")
                        for j in range(4):
                            nc.tensor.matmul(
                                vps[:],
                                xt[:, 2 * j : 2 * j + 2, ti * 128 : ti * 128 + 128],
                                wqkv[
                                    :, 2 * j : 2 * j + 2,
                                    2 * D + half * 512 : 2 * D + half * 512 + 512,
                                ],
                                start=(j == 0),
                                stop=(j == 3),
                                perf_mode=DR,
                            )
                        if kv_bias_zero:
                            nc.scalar.copy(
                                vst[:, ti, half * 512 : half * 512 + 512], vps[:]
                            )
                        else:
                            nc.vector.tensor_tensor(
                                out=vst[:, ti, half * 512 : half * 512 + 512],
                                in0=vps[:],
                                in1=bv_row[:, half * 512 : half * 512 + 512],
                                op=mybir.AluOpType.add,
                            )
                # re-chunk V to 120-strided AV tiles
                vav = vavp.tile([128, 4, D], BF16, tag="vav")
                for j, jw in jl:
                    r0 = 120 * j
                    c0 = r0 // 128
                    o0 = r0 - c0 * 128
                    n1 = 128 - o0
                    nc.sync.dma_start(out=vav[0:n1, j, :], in_=vst[o0:128, c0, :])
                    if n1 < 128:
                        nc.sync.dma_start(
                            out=vav[n1:128, j, :], in_=vst[0 : 128 - n1, c0 + 1, :]
                        )
                state[("kt", b, ci)] = kt
                state[("vav", b, ci)] = vav

            def attend_a(b, ci):
                ws, wcnt, ss, sl, ntc, jl = CHUNKS[ci]
                qt = state[("qt", b)]
                kt = state.pop(("kt", b, ci))

                # scores per head + band gather
                slk = SLK[ci]
                band = smx.tile([wcnt, 8, T], F32, tag="band")
                for h in range(8):
                    sps = sc_ps.tile([wcnt, slk], F32, tag="sps")
                    nc.tensor.matmul(
                        sps[:], qt[:, h, ws : ws + wcnt], kt[:, h, :],
                        start=True, stop=True,
                    )
                    ssb = ssbp.tile([wcnt, slk], F32, tag="ssb")
                    if h % 2 == 0:
                        nc.scalar.copy(ssb[:], sps[:])
                    else:
                        nc.vector.tensor_copy(ssb[:], sps[:])
                    rs = _rowsz(ssb)
                    nc.sync.dma_start(
                        out=band[:, h, :],
                        in_=_flat_ap(ssb, 0, [[rs + 5, wcnt], [1, T]]),
                    )

                # softmax in band domain
                negmax = smx.tile([wcnt, 8], F32, tag="negmax")
                nc.vector.tensor_reduce(
                    negmax[:], band[:], axis=mybir.AxisListType.X,
                    op=mybir.AluOpType.max, negate=True,
                )
                negmax_s = smx.tile([wcnt, 8], F32, tag="negmax_s")
                nc.vector.tensor_scalar_mul(negmax_s[:], negmax[:], SSCALE)
                eb = smx.tile([wcnt, 8, T], F32, tag="eb")
                sums = smx.tile([wcnt, 8], F32, tag="sums")
                for h in range(8):
                    nc.scalar.activation(
                        eb[:, h, :], band[:, h, :],
                        mybir.ActivationFunctionType.Exp,
                        bias=negmax_s[:, h : h + 1], scale=SSCALE,
                        accum_out=sums[:, h : h + 1],
                    )
                # v is stored at WS8x scale; normalize attn by WS8*sums instead
                sums2 = smx.tile([wcnt, 8], F32, tag="sums2")
                nc.vector.tensor_scalar_mul(sums2[:], sums[:], WS8)
                recip = smx.tile([wcnt, 8], F32, tag="recip")
                nc.vector.reciprocal(recip[:], sums2[:])
                attn = smx.tile([wcnt, 8, T], BF16, tag="attn")
                recip_bc = recip[:].unsqueeze(2).to_broadcast([wcnt, 8, T])
                nc.vector.tensor_tensor(
                    out=attn[:], in0=eb[:], in1=recip_bc, op=mybir.AluOpType.mult
                )

                # scatter attn into transposed sheared DRAM staging
                # Ash_dT[row = h*96 + j*24 + w', col = 5w' + t]; t contiguous
                ash_dT = drp.tile([768, 128], BF16, tag="ashd")
                nc.sync.dma_start(out=ash_dT[:], in_=zash[:])
                ra = _rowsz(attn)
                rdt = _rowsz(ash_dT)
                for j, jw in jl:
                    if jw == 24:
                        nc.sync.dma_start(
                            out=_flat_ap(
                                ash_dT, 24 * j * rdt,
                                [[rdt + 5, 24], [96 * rdt, 8], [1, T]],
                            ),
                            in_=_flat_ap(
                                attn, 24 * j * ra, [[ra, 24], [T, 8], [1, T]]
                            ),
                        )
                    else:
                        nc.sync.dma_start(
                            out=_flat_ap(
                                ash_dT, 24 * j * rdt, [[96 * rdt, 8], [1, T]]
                            ),
                            in_=attn[24 * j : 24 * j + 1, :, :],
                        )
                state[("ashd", b, ci)] = ash_dT

            def attend_b(b, ci):
                ws, wcnt, ss, sl, ntc, jl = CHUNKS[ci]
                vav = state.pop(("vav", b, ci))
                oT = state[("oT", b)]
                ash_dT = state.pop(("ashd", b, ci))
                # contiguous reload as [128, 6, 128], PE-transpose to [128, 768]
                ashT = ashp.tile([128, 6, 128], BF16, tag="ashT")
                nc.sync.dma_start(
                    out=ashT, in_=ash_dT[:].rearrange("(blk p) c -> p blk c", p=128)
                )
                ash = ashp.tile([128, 8 * 4 * 24], BF16, tag="ash")
                for blk in range(6):
                    tps = tr_ps.tile([128, 128], BF16, tag="ashtp")
                    nc.tensor.transpose(tps[:], ashT[:, blk, :], ident[:])
                    nc.vector.tensor_copy(ash[:, blk * 128 : blk * 128 + 128], tps[:])

                # AV: oT[d, w'] per (h, j), V stationary
                avo = av_ps.tile([128, 4, 256], F32, tag="avo")
                for j, jw in jl:
                    for h in range(8):
                        nc.tensor.matmul(
                            avo[:, j, h * 24 : h * 24 + jw],
                            vav[:, j, h * 128 : h * 128 + 128],
                            ash[:, h * 96 + j * 24 : h * 96 + j * 24 + jw],
                            start=True,
                            stop=True,
                        )
                # evict into per-batch oT at col h*NWQ + ws + 24j + w'
                if ci < 4:
                    nc.vector.tensor_copy(
                        oT[:, :, ws : ws + 96].rearrange("p h (j w) -> p j h w", j=4),
                        avo[:, :, 0:192].rearrange("p j (h w) -> p j h w", h=8),
                    )
                else:
                    nc.vector.tensor_copy(
                        oT[:, :, ws : ws + 24],
                        avo[:, 0, 0:192].rearrange("p (h w) -> p h w", h=8),
                    )
                    nc.vector.tensor_copy(
                        oT[:, :, ws + 24 : ws + 25],
                        avo[:, 1, 0:192].rearrange("p (h w) -> p h w", h=8)[:, :, 0:1],
                    )

            def tail(b):
                """out_proj + head + log_softmax + output DMA for one batch."""
                oT = state.pop(("oT", b))
                state.pop(("qt", b))
                featT = perb1.tile([128, 8, NWQ], BF16, tag="featT")
                for fc in range(8):
                    fps = proj_ps.tile([128, NWQ], F32, tag="pps")
                    for kc in range(8):
                        nc.tensor.matmul(
                            fps[:],
                            wo[:, kc, fc * 128 : fc * 128 + 128],
                            oT[:, kc, :],
                            start=(kc == 0),
                            stop=(kc == 7),
                        )
                    nc.vector.tensor_scalar_add(
                        featT[:, fc, :], fps[:], bo_col[:, fc : fc + 1]
                    )

                pps = proj_ps.tile([NC, NWQ], F32, tag="pps")
                for kc in range(8):
                    nc.tensor.matmul(
                        pps[:], whead[:, kc, :], featT[:, kc, :],
                        start=(kc == 0), stop=(kc == 7),
                    )
                probsT = perb2.tile([NC, NW], F32, tag="probsT")
                nc.vector.tensor_scalar_add(probsT[:], pps[:, 0:NW], bhead_col[:])

                # transpose probs to token-major [128, 4, 2] via PE
                ptm = perb2.tile([128, 4, NC], F32, tag="ptm")
                nc.vector.memset(ptm, 0.0)
                for wc in range(4):
                    cnt = 128 if wc < 3 else 25
                    tp2 = tr_ps.tile([128, NC], F32, tag="ashtp")
                    nc.tensor.transpose(
                        tp2[0:cnt, :], probsT[:, wc * 128 : wc * 128 + cnt],
                        ident_f[0:NC, 0:NC],
                    )
                    nc.vector.tensor_copy(ptm[0:cnt, wc, :], tp2[0:cnt, :])

                # log_softmax over the 2 classes
                pmax = perb2.tile([128, 4], F32, tag="pmax")
                nc.vector.tensor_reduce(
                    pmax[:], ptm[:], axis=mybir.AxisListType.X, op=mybir.AluOpType.max
                )
                nmax = perb2.tile([128, 4], F32, tag="nmax")
                nc.vector.tensor_scalar_mul(nmax[:], pmax[:], -1.0)
                sexp = perb2.tile([128, 4], F32, tag="sexp")
                etmp = perb2.tile([128, 4, NC], F32, tag="etmp")
                for wc in range(4):
                    nc.scalar.activation(
                        etmp[:, wc, :], ptm[:, wc, :],
                        mybir.ActivationFunctionType.Exp,
                        bias=nmax[:, wc : wc + 1], scale=1.0,
                        accum_out=sexp[:, wc : wc + 1],
                    )
                lns = perb2.tile([128, 4], F32, tag="lns")
                nc.scalar.activation(lns[:], sexp[:], mybir.ActivationFunctionType.Ln)
                otm = perb2.tile([128, 4, NC], F32, tag="otm")
                for wc in range(4):
                    nc.vector.tensor_scalar(
                        out=otm[:, wc, :],
                        in0=ptm[:, wc, :],
                        scalar1=pmax[:, wc : wc + 1],
                        scalar2=lns[:, wc : wc + 1],
                        op0=mybir.AluOpType.subtract,
                        op1=mybir.AluOpType.subtract,
                    )

                # write output: const fill + strided scatter
                nc.sync.dma_start(
                    out=bass.AP(
                        tensor=out_d, offset=b * L * NC + 2,
                        ap=[[1, 1], [10, NW], [1, 8]],
                    ),
                    in_=_flat_ap(cfill, 0, [[_rowsz(cfill), 1], [0, NW], [1, 8]]),
                )
                nc.sync.dma_start(
                    out=bass.AP(
                        tensor=out_d, offset=b * L * NC + 2045 * NC, ap=[[1, 6]]
                    ),
                    in_=cfill[0:1, 0:6],
                )
                rt = _rowsz(otm)
                nc.sync.dma_start(
                    out=bass.AP(
                        tensor=out_d, offset=b * L * NC,
                        ap=[[10, 128], [1280, 3], [1, NC]],
                    ),
                    in_=_flat_ap(otm, 0, [[rt, 128], [NC, 3], [1, NC]]),
                )
                nc.sync.dma_start(
                    out=bass.AP(
                        tensor=out_d, offset=b * L * NC + 3840, ap=[[10, 25], [1, NC]]
                    ),
                    in_=_flat_ap(otm, 3 * NC, [[rt, 25], [1, NC]]),
                )

            def start_batch(b):
                qproj(b)
                oT = perb.tile([128, 8, NWQ], BF16, tag="oT")
                nc.vector.memset(oT[:, :, NW:NWQ], 0.0)
                state[("oT", b)] = oT

            # ---- software-pipelined emission (2-deep) ----
            iters = [(b, ci) for b in range(BL) for ci in range(5)]
            start_batch(0)
            produce(0, 0)
            attend_a(0, 0)
            for i in range(1, len(iters)):
                b, ci = iters[i]
                if ci == 0:
                    start_batch(b)
                produce(b, ci)
                attend_a(b, ci)
                pb, pci = iters[i - 1]
                attend_b(pb, pci)
                if pci == 4:
                    tail(pb)
            lb, lci = iters[-1]
            attend_b(lb, lci)
            tail(lb)

    nc.compile()
    return nc


_NC_CACHE = {}


def _get_nc(kv_bias_zero):
    if kv_bias_zero not in _NC_CACHE:
        _NC_CACHE[kv_bias_zero] = _build(kv_bias_zero)
    return _NC_CACHE[kv_bias_zero]


def kernel(x, in_proj_w, in_proj_b, out_proj_w, out_proj_b, out_w, out_b, x_len=None,
           _want_perf=False):
    x = np.asarray(x, dtype=np.float32)
    in_proj_w = np.asarray(in_proj_w, dtype=np.float32)
    in_proj_b = np.asarray(in_proj_b, dtype=np.float32)
    out_proj_w = np.asarray(out_proj_w, dtype=np.float32)
    out_proj_b = np.asarray(out_proj_b, dtype=np.float32)
    out_w = np.asarray(out_w, dtype=np.float32)
    out_b = np.asarray(out_b, dtype=np.float32)

    kv_bias_zero = bool(np.all(in_proj_b[D:] == 0.0))
    nc = _get_nc(kv_bias_zero)

    # host-side layout prep; x and in_proj weights go to fp8 (e4m3) for
    # DoubleRow matmuls, with weights pre-scaled by WS8 to center the
    # small-magnitude weight distribution in e4m3's normal range.
    xt = np.zeros((B, D, LP), dtype=ml_dtypes.float8_e4m3)
    xt[:, :, :L] = x.transpose(0, 2, 1).astype(ml_dtypes.float8_e4m3)
    xq = np.zeros((B, D, NWQ), dtype=ml_dtypes.float8_e4m3)
    xq[:, :, :NW] = xt[:, :, 5 : 5 * NW + 5 : 5]
    wqkvt = np.ascontiguousarray(in_proj_w.T * np.float32(WS8)).astype(
        ml_dtypes.float8_e4m3
    )
    wot = np.ascontiguousarray(out_proj_w.T).astype(ml_dtypes.bfloat16)
    wheadt = np.ascontiguousarray(out_w.T).astype(ml_dtypes.bfloat16)

    in_maps = []
    for c in range(NCORES):
        in_maps.append({
            "xt": np.ascontiguousarray(xt[c * BL : (c + 1) * BL]),
            "xq": np.ascontiguousarray(xq[c * BL : (c + 1) * BL]),
            "wqkvt": wqkvt,
            "wot": wot,
            "wheadt": wheadt,
            "bqkv": in_proj_b * np.float32(WS8),
            "bo": out_proj_b,
            "bhead": out_b,
        })

    kr = run_bass_kernel_spmd(
        nc, in_maps, core_ids=list(range(NCORES)), trace=_want_perf
    )
    out = np.concatenate([r["out"] for r in kr.results], axis=0).reshape(-1, NC)
    if _want_perf:
        return out, kr
    return out



# revision 33
# speedup vs baseline: 2.0976x; 1.5580x over previous
"""Trainium2 Bass kernel for nn_ExpWindowAttention (windowed sparse attention).

Strategy: pure data-parallel over batch (32 -> 8 cores x 4 batches).

Key algebraic fusion: only probs = (o @ wo.T + bo) @ wh.T + bh is consumed
(nC=2), so the whole post-softmax pipeline collapses to rank-2 per head:
  probs[w, c] = sum_{h,t} attn[w, h, t] * vp[5w + t, h, c] + BC[c]
  vp = x @ wvp.T,  wvp[(h, c), :] = (wh @ wo)[c, hslice] @ wv[hslice, :]
  BC = wh @ (wo @ bv + bo) + bh
This eliminates the V projection, attention-times-V, out-projection, and
head GEMMs entirely; vp is a 16-wide projection and the banded contraction
runs on the vector engine.

Q/K projections run in fp8 (e4m3) with DoubleRow matmuls (2 contraction
rows/cycle). Weights are pre-scaled by 128 into e4m3's normal range; the
resulting 128^2 score scale folds into the softmax exp scale. vp uses
bf16 weights against the fp8 x for accuracy (it feeds probs directly).

Per (batch, window-chunk): project K from a position segment, compute
dense per-head scores, extract the 11-wide diagonal band with a flat-AP
DMA gather, softmax in the compact band domain, band-gather vp and
contract attn x vp on the vector engine, then log_softmax + scatter.
"""

import numpy as np
import ml_dtypes

import concourse.bass as bass
from concourse import bacc
import concourse.mybir as mybir
import concourse.tile as tile
from concourse.bass_utils import run_bass_kernel_spmd

F32 = mybir.dt.float32
BF16 = mybir.dt.bfloat16
F8 = mybir.dt.float8e4
DR = mybir.MatmulPerfMode.DoubleRow
WS8 = 128.0              # fp8 weight pre-scale for q/k projections

NCORES = 8
B = 32
BL = B // NCORES          # batches per core
L = 2048
D = 1024
H = 8
HD = 128
W = 5
T = 2 * W + 1             # 11
NW = 409                  # windows per batch
NWQ = 416                 # padded center count (32-mult)
NC = 2
LP = 2176                 # padded position count (17*128)
SCALE = float(1.0 / np.sqrt(HD))
SSCALE = float(SCALE / (WS8 * WS8))   # scores carry WS8^2 from fp8 q/k scaling
NEGLOG2 = float(-np.log(2.0))

# (window_start, window_count, seg_start, seg_len)
CHUNKS = []
for c in range(5):
    if c < 4:
        CHUNKS.append((96 * c, 96, 480 * c, 512))
    else:
        CHUNKS.append((96 * c, 25, 1920, 256))
SLK = [512, 512, 512, 512, 136]  # K/scores width per chunk (tail trimmed)


def _flat_ap(t, extra_offset, dims):
    """AP over a tile's backing tensor flat element space (partition-major)."""
    return bass.AP(tensor=t.tensor, offset=t.offset + extra_offset,
                   ap=[list(d) for d in dims])


def _rowsz(t):
    """True per-partition stride (elements) of a tile, from its own AP."""
    return int(t[:].ap[0][0])


def _build(debug=False):
    nc = bacc.Bacc(None, target_bir_lowering=False)

    xt_d = nc.declare_dram_parameter("xt", [BL, D, LP], F8, isOutput=False)
    xq_d = nc.declare_dram_parameter("xq", [BL, D, NWQ], F8, isOutput=False)
    wqk_d = nc.declare_dram_parameter("wqkt", [D, 2 * D], F8, isOutput=False)
    wvp_d = nc.declare_dram_parameter("wvpt", [D, 2 * H], BF16, isOutput=False)
    bqk_d = nc.declare_dram_parameter("bqk", [2 * D], F32, isOutput=False)
    bc2_d = nc.declare_dram_parameter("bc2", [NC], F32, isOutput=False)
    out_d = nc.declare_dram_parameter("out", [BL, L, NC], F32, isOutput=True)
    if debug:
        dbg_vps = nc.declare_dram_parameter("dbg_vps", [2 * H, 512], F32, isOutput=True)
        dbg_vpb = nc.declare_dram_parameter("dbg_vpb", [96, NC, 8, T], F32, isOutput=True)
        dbg_attn = nc.declare_dram_parameter("dbg_attn", [96, 8, T], F32, isOutput=True)
        dbg_pb = nc.declare_dram_parameter("dbg_pb", [96, 5, NC], F32, isOutput=True)
        dbg_kt = nc.declare_dram_parameter("dbg_kt", [128, 8, 512], BF16, isOutput=True)

    with tile.TileContext(nc) as tc:
        import contextlib
        with contextlib.ExitStack() as ctx:
            const = ctx.enter_context(tc.tile_pool(name="const", bufs=1))
            perb = ctx.enter_context(tc.tile_pool(name="perb", bufs=2))
            perb1 = ctx.enter_context(tc.tile_pool(name="perb1", bufs=1))
            perb2 = ctx.enter_context(tc.tile_pool(name="perb2", bufs=2))
            xtp = ctx.enter_context(tc.tile_pool(name="xtp", bufs=2))
            ktp = ctx.enter_context(tc.tile_pool(name="ktp", bufs=2))
            vpsp = ctx.enter_context(tc.tile_pool(name="vpsp", bufs=2))
            drp = ctx.enter_context(tc.tile_pool(name="drp", bufs=2, space="DRAM"))
            ssbp = ctx.enter_context(tc.tile_pool(name="ssbp", bufs=3))
            smx = ctx.enter_context(tc.tile_pool(name="smx", bufs=2))
            proj_ps = ctx.enter_context(tc.tile_pool(name="proj_ps", bufs=2, space="PSUM"))
            vp_ps = ctx.enter_context(tc.tile_pool(name="vp_ps", bufs=2, space="PSUM"))
            sc_ps = ctx.enter_context(tc.tile_pool(name="sc_ps", bufs=3, space="PSUM"))

            # ---- resident weights / biases ----
            wqk = const.tile([128, 8, 2 * D], F8)
            nc.sync.dma_start(
                out=wqk, in_=wqk_d.rearrange("(kc p) c -> p kc c", p=128)
            )
            wvp = const.tile([128, 8, 2 * H], BF16)
            nc.sync.dma_start(
                out=wvp, in_=wvp_d.rearrange("(kc p) c -> p kc c", p=128)
            )
            bqk_col = const.tile([128, 16], F32)  # [p, proj*8+fc]
            nc.sync.dma_start(out=bqk_col, in_=bqk_d.rearrange("(c p) -> p c", p=128))
            bias2 = const.tile([128, NC], F32)    # BC broadcast along partitions
            nc.sync.dma_start(
                out=bias2, in_=bass.AP(tensor=bc2_d, offset=0, ap=[[0, 128], [1, NC]])
            )
            cfill = const.tile([128, 8], F32)
            nc.vector.memset(cfill, NEGLOG2)

            state = {}

            def qproj(b):
                """Per-batch Q projection from host-pregathered centers."""
                xq = perb1.tile([128, 8, NWQ], F8, tag="xq")
                nc.sync.dma_start(
                    out=xq, in_=xq_d[b].rearrange("(kc p) c -> p kc c", p=128)
                )
                qt = perb.tile([128, 8, NWQ], BF16, tag="qt")
                for h in range(8):
                    qps = proj_ps.tile([128, NWQ], F32, tag="pps")
                    for j in range(4):
                        nc.tensor.matmul(
                            qps[:],
                            wqk[:, 2 * j : 2 * j + 2, h * 128 : h * 128 + 128],
                            xq[:, 2 * j : 2 * j + 2, :],
                            start=(j == 0),
                            stop=(j == 3),
                            perf_mode=DR,
                        )
                    nc.vector.tensor_scalar_add(
                        qt[:, h, :], qps[:], bqk_col[:, h : h + 1]
                    )
                state[("qt", b)] = qt

            def produce(b, ci):
                """K projection + vp (rank-2 value path) for one (batch, chunk)."""
                ws, wcnt, ss, sl = CHUNKS[ci]
                xt = xtp.tile([128, 8, sl], F8, tag="xt")
                nc.sync.dma_start(
                    out=xt,
                    in_=xt_d[b].rearrange("(kc p) c -> p kc c", p=128)[
                        :, :, ss : ss + sl
                    ],
                )
                slk = SLK[ci]
                kt = ktp.tile([128, 8, slk], BF16, tag="kt")
                for h in range(8):
                    kps = proj_ps.tile([128, slk], F32, tag="pps")
                    for j in range(4):
                        nc.tensor.matmul(
                            kps[:],
                            wqk[:, 2 * j : 2 * j + 2, D + h * 128 : D + h * 128 + 128],
                            xt[:, 2 * j : 2 * j + 2, 0:slk],
                            start=(j == 0),
                            stop=(j == 3),
                            perf_mode=DR,
                        )
                    nc.vector.tensor_scalar_add(
                        kt[:, h, :], kps[:], bqk_col[:, 8 + h : 9 + h]
                    )
                # vp chunk: [16, sl] = wvp.T @ x  (bf16 weights x fp8 x),
                # staged to DRAM as [c, pos, h] so the band read is contiguous
                vp_d = state[("vp_d", b)]
                vps = vp_ps.tile([2 * H, sl], F32, tag="vps")
                for kc in range(8):
                    nc.tensor.matmul(
                        vps[:],
                        wvp[:, kc, :],
                        xt[:, kc, :],
                        start=(kc == 0),
                        stop=(kc == 7),
                    )
                vpsb = vpsp.tile([2 * H, sl], F32, tag="vpsb")
                nc.vector.tensor_copy(vpsb[:], vps[:])
                rsv = _rowsz(vpsb)
                for c in range(NC):
                    nc.sync.dma_start(
                        out=_flat_ap(
                            vp_d, c * 8 * LP + ss, [[LP, 8], [1, sl]]
                        ),
                        in_=_flat_ap(vpsb, c * rsv, [[2 * rsv, 8], [1, sl]]),
                    )
                if debug and b == 0 and ci == 0:
                    nc.sync.dma_start(out=dbg_vps[:, :], in_=vpsb[:])
                    nc.sync.dma_start(out=dbg_kt[:, :, :], in_=kt[:])
                state[("kt", b, ci)] = kt

            def attend(b, ci):
                ws, wcnt, ss, sl = CHUNKS[ci]
                qt = state[("qt", b)]
                kt = state.pop(("kt", b, ci))

                # scores per head + band gather
                slk = SLK[ci]
                band = smx.tile([wcnt, 8, T], F32, tag="band")
                for h in range(8):
                    sps = sc_ps.tile([wcnt, slk], F32, tag="sps")
                    nc.tensor.matmul(
                        sps[:], qt[:, h, ws : ws + wcnt], kt[:, h, :],
                        start=True, stop=True,
                    )
                    ssb = ssbp.tile([wcnt, slk], F32, tag="ssb")
                    if h % 2 == 0:
                        nc.scalar.copy(ssb[:], sps[:])
                    else:
                        nc.vector.tensor_copy(ssb[:], sps[:])
                    rs = _rowsz(ssb)
                    nc.sync.dma_start(
                        out=band[:, h, :],
                        in_=_flat_ap(ssb, 0, [[rs + 5, wcnt], [1, T]]),
                    )

                # softmax in band domain
                negmax = smx.tile([wcnt, 8], F32, tag="negmax")
                nc.vector.tensor_reduce(
                    negmax[:], band[:], axis=mybir.AxisListType.X,
                    op=mybir.AluOpType.max, negate=True,
                )
                negmax_s = smx.tile([wcnt, 8], F32, tag="negmax_s")
                nc.vector.tensor_scalar_mul(negmax_s[:], negmax[:], SSCALE)
                eb = smx.tile([wcnt, 8, T], F32, tag="eb")
                sums = smx.tile([wcnt, 8], F32, tag="sums")
                for h in range(8):
                    nc.scalar.activation(
                        eb[:, h, :], band[:, h, :],
                        mybir.ActivationFunctionType.Exp,
                        bias=negmax_s[:, h : h + 1], scale=SSCALE,
                        accum_out=sums[:, h : h + 1],
                    )
                recip = smx.tile([wcnt, 8], F32, tag="recip")
                nc.vector.reciprocal(recip[:], sums[:])
                attn = smx.tile([wcnt, 8, T], F32, tag="attn")
                recip_bc = recip[:].unsqueeze(2).to_broadcast([wcnt, 8, T])
                nc.vector.tensor_tensor(
                    out=attn[:], in0=eb[:], in1=recip_bc, op=mybir.AluOpType.mult
                )

                # banded value contraction: probs[w, c] = sum_{h,t} attn * vp
                vp_d = state[("vp_d", b)]
                vpb = smx.tile([wcnt, NC, 8, T], F32, tag="vpb")
                for c in range(NC):
                    nc.sync.dma_start(
                        out=vpb[:, c, :, :],
                        in_=_flat_ap(
                            vp_d, 480 * ci + c * 8 * LP,
                            [[5, wcnt], [LP, 8], [1, T]],
                        ),
                    )
                prod = smx.tile([wcnt, NC, 8, T], F32, tag="prod")
                attn_bc = attn[:].unsqueeze(1).to_broadcast([wcnt, NC, 8, T])
                nc.vector.tensor_tensor(
                    out=prod[:], in0=vpb[:], in1=attn_bc, op=mybir.AluOpType.mult
                )
                red1 = smx.tile([wcnt, NC, 8], F32, tag="red1")
                nc.vector.tensor_reduce(
                    red1[:], prod[:], axis=mybir.AxisListType.X,
                    op=mybir.AluOpType.add,
                )
                pb = state[("pb", b)]
                nc.vector.tensor_reduce(
                    pb[0:wcnt, ci, :], red1[:], axis=mybir.AxisListType.X,
                    op=mybir.AluOpType.add,
                )
                if debug and b == 0 and ci == 0:
                    nc.sync.dma_start(out=dbg_vpb[:, :, :, :], in_=vpb[:])
                    nc.sync.dma_start(out=dbg_attn[:, :, :], in_=attn[:])

            def tail(b):
                """bias + log_softmax + output DMA for one batch."""
                pb = state.pop(("pb", b))
                state.pop(("qt", b))
                state.pop(("vp_d", b))
                if debug and b == 0:
                    nc.sync.dma_start(out=dbg_pb[:, :, :], in_=pb[:])
                pb2 = perb2.tile([96, 5, NC], F32, tag="pb2")
                bias_bc = bias2[0:96, :].unsqueeze(1).to_broadcast([96, 5, NC])
                nc.vector.tensor_tensor(
                    out=pb2[:], in0=pb[:], in1=bias_bc, op=mybir.AluOpType.add
                )

                pmax = perb2.tile([96, 5], F32, tag="pmax")
                nc.vector.tensor_reduce(
                    pmax[:], pb2[:], axis=mybir.AxisListType.X, op=mybir.AluOpType.max
                )
                nmax = perb2.tile([96, 5], F32, tag="nmax")
                nc.vector.tensor_scalar_mul(nmax[:], pmax[:], -1.0)
                sexp = perb2.tile([96, 5], F32, tag="sexp")
                etmp = perb2.tile([96, 5, NC], F32, tag="etmp")
                for wc in range(5):
                    nc.scalar.activation(
                        etmp[:, wc, :], pb2[:, wc, :],
                        mybir.ActivationFunctionType.Exp,
                        bias=nmax[:, wc : wc + 1], scale=1.0,
                        accum_out=sexp[:, wc : wc + 1],
                    )
                lns = perb2.tile([96, 5], F32, tag="lns")
                nc.scalar.activation(lns[:], sexp[:], mybir.ActivationFunctionType.Ln)
                otm = perb2.tile([96, 5, NC], F32, tag="otm")
                for wc in range(5):
                    nc.vector.tensor_scalar(
                        out=otm[:, wc, :],
                        in0=pb2[:, wc, :],
                        scalar1=pmax[:, wc : wc + 1],
                        scalar2=lns[:, wc : wc + 1],
                        op0=mybir.AluOpType.subtract,
                        op1=mybir.AluOpType.subtract,
                    )

                # write output: const fill + strided scatter
                nc.sync.dma_start(
                    out=bass.AP(
                        tensor=out_d, offset=b * L * NC + 2,
                        ap=[[1, 1], [10, NW], [1, 8]],
                    ),
                    in_=_flat_ap(cfill, 0, [[_rowsz(cfill), 1], [0, NW], [1, 8]]),
                )
                nc.sync.dma_start(
                    out=bass.AP(
                        tensor=out_d, offset=b * L * NC + 2045 * NC, ap=[[1, 6]]
                    ),
                    in_=cfill[0:1, 0:6],
                )
                rt = _rowsz(otm)
                nc.sync.dma_start(
                    out=bass.AP(
                        tensor=out_d, offset=b * L * NC,
                        ap=[[10, 96], [960, 4], [1, NC]],
                    ),
                    in_=_flat_ap(otm, 0, [[rt, 96], [NC, 4], [1, NC]]),
                )
                nc.sync.dma_start(
                    out=bass.AP(
                        tensor=out_d, offset=b * L * NC + 3840, ap=[[10, 25], [1, NC]]
                    ),
                    in_=_flat_ap(otm, 4 * NC, [[rt, 25], [1, NC]]),
                )

            def start_batch(b):
                qproj(b)
                state[("vp_d", b)] = drp.tile(
                    [NC, 8, LP], F32, tag="vp_d", name="vp_d"
                )
                state[("pb", b)] = perb.tile([96, 5, NC], F32, tag="pb", name="pb")

            # ---- software-pipelined emission (2-deep) ----
            iters = [(b, ci) for b in range(BL) for ci in range(5)]
            start_batch(0)
            produce(0, 0)
            for i in range(1, len(iters)):
                b, ci = iters[i]
                if ci == 0:
                    start_batch(b)
                produce(b, ci)
                pb_, pci = iters[i - 1]
                attend(pb_, pci)
                if pci == 4:
                    tail(pb_)
            lb, lci = iters[-1]
            attend(lb, lci)
            tail(lb)

    nc.compile()
    return nc


_NC_CACHE = {}


def _get_nc(debug=False):
    if debug not in _NC_CACHE:
        _NC_CACHE[debug] = _build(debug)
    return _NC_CACHE[debug]


def kernel(x, in_proj_w, in_proj_b, out_proj_w, out_proj_b, out_w, out_b, x_len=None,
           _want_perf=False, _debug=False):
    x = np.asarray(x, dtype=np.float32)
    in_proj_w = np.asarray(in_proj_w, dtype=np.float32)
    in_proj_b = np.asarray(in_proj_b, dtype=np.float32)
    out_proj_w = np.asarray(out_proj_w, dtype=np.float32)
    out_proj_b = np.asarray(out_proj_b, dtype=np.float32)
    out_w = np.asarray(out_w, dtype=np.float32)
    out_b = np.asarray(out_b, dtype=np.float32)

    nc = _get_nc(_debug)

    # host-side layout prep; x and q/k weights go to fp8 (e4m3) for
    # DoubleRow matmuls, with weights pre-scaled by WS8 to center the
    # small-magnitude weight distribution in e4m3's normal range.
    xt = np.zeros((B, D, LP), dtype=ml_dtypes.float8_e4m3)
    xt[:, :, :L] = x.transpose(0, 2, 1).astype(ml_dtypes.float8_e4m3)
    xq = np.zeros((B, D, NWQ), dtype=ml_dtypes.float8_e4m3)
    xq[:, :, :NW] = xt[:, :, 5 : 5 * NW + 5 : 5]
    wqkt = np.ascontiguousarray(
        in_proj_w[: 2 * D].T * np.float32(WS8)
    ).astype(ml_dtypes.float8_e4m3)

    # fused rank-2 value path (computed in float64 host-side):
    # wvp[(h,c), :] = (wh @ wo)[c, hslice] @ wv[hslice, :]
    wv = in_proj_w[2 * D :].astype(np.float64)
    bv = in_proj_b[2 * D :].astype(np.float64)
    wo64 = out_proj_w.astype(np.float64)
    wh64 = out_w.astype(np.float64)
    wf = wh64 @ wo64                      # [2, D]
    wvp = np.zeros((2 * H, D), np.float64)
    for h in range(H):
        sl_ = slice(h * HD, (h + 1) * HD)
        wvp[2 * h : 2 * h + 2] = wf[:, sl_] @ wv[sl_, :]
    bc2 = (wh64 @ (wo64 @ bv + out_proj_b.astype(np.float64))
           + out_b.astype(np.float64)).astype(np.float32)
    wvpt = np.ascontiguousarray(wvp.T).astype(ml_dtypes.bfloat16)

    in_maps = []
    for c in range(NCORES):
        in_maps.append({
            "xt": np.ascontiguousarray(xt[c * BL : (c + 1) * BL]),
            "xq": np.ascontiguousarray(xq[c * BL : (c + 1) * BL]),
            "wqkt": wqkt,
            "wvpt": wvpt,
            "bqk": in_proj_b[: 2 * D] * np.float32(WS8),
            "bc2": bc2,
        })

    if _debug:
        kr = run_bass_kernel_spmd(nc, in_maps[:1], core_ids=[0])
        return kr.results[0]
    kr = run_bass_kernel_spmd(
        nc, in_maps, core_ids=list(range(NCORES)), trace=_want_perf
    )
    out = np.concatenate([r["out"] for r in kr.results], axis=0).reshape(-1, NC)
    if _want_perf:
        return out, kr
    return out
